# revision 47
# baseline (speedup 1.0000x reference)
"""Involution kernel for Trainium2, 8 NeuronCores.

Sharding: data-parallel over (batch=4) x (H halves=2) -> 8 shards of
28 output rows each, with a 3-row halo (K=7 unfold). Host zero-pads the
image to 62 cols so every shard is a uniform [256, 36, 62] slab in a
flat "padded raster" layout (q = r*62 + w), stored fp16 twice (even and
odd-shifted copies) so every tap's x view is 4B-aligned for DVE 2x mode.

Per-core pipeline, channel-major, pixels PACKED to 28x56=1568 (padded
columns are never computed on; x reads use 3D row-strided APs). Work is
split into 4 quadrants (2 pixel halves x 2 channel tiles) of 49 tap
units each:
  A. reduce conv (1x1, BN+ReLU folded on host) -> r [128, 1568] fp16
  B. per unit: span conv emits per-channel dynamic weights via a
     host-replicated [128,128] fp16 weight block (one 2-bank PSUM tile,
     2 matmuls of 392 cols). The tap product (wm + b2) * x_shift is
     formed by one of three engine paths, LP-balanced so all four
     engines carry ~28us per quadrant (GPSIMD cannot touch PSUM on
     real silicon, so every PSUM read is ScalarE/VectorE):
       - D (17/49): VectorE scalar_tensor_tensor straight from PSUM,
         fusing bias add + multiply (942ns, no extract needed)
       - P (16/49): ScalarE extracts wm+b2 to SBUF fp16 (838ns), Pool
         tensor_tensor multiplies (1651ns)
       - A (16/49): ScalarE extracts, VectorE multiplies in fp16 at
         DVE 2x rate (469ns), emitted a few units late so the in-order
         DVE queue never head-of-line blocks on ScalarE
     repeating in D,P,A order (measured fastest interleave by ~3us).
     Tap accumulation: 25/49 products go straight to PE (identity
     matmul into a 2-bank PSUM accumulator, software-pipelined ~12
     units behind), 24/49 are first summed in pairs on VectorE fp16
     (469ns halves the PE merge work) and the pairsums PE-merged.
  C. per quadrant: ScalarE copies the PSUM accumulator out with +49*eps
     folded into the bias, then DMA; copy+DMA are deferred into the
     next quadrant so the ScalarE queue is never blocked at the
     boundary. The last two units of each quadrant use the D path with
     direct PE accumulation (shortest dependency chain) and the PE
     window tapers so the tail drain never serializes.

Prologue: every dma_start costs its issuing sequencer ~0.9-1.3us, and
the ScalarE queue must reach the phase A relu fast (it gates the first
span and the whole VectorE pipe), so the scalar queue carries only the
two earliest-needed loads (xhe tile-1 chunk A, b2) and everything else
rides SP (DMA_PLAN=2). The x slabs stream in two row chunks so phase A
starts before the halo rows land.
"""

import sys
import numpy as np

for _p in ("/opt/trn_rl_repo",):
    if _p not in sys.path:
        sys.path.insert(0, _p)

import concourse.bass as bass
import concourse.tile as tile
from concourse import mybir
from concourse.bass_utils import run_bass_kernel_spmd
import bass_rust

F32 = mybir.dt.float32
F16 = mybir.dt.float16

N_CORES = 8
C = 256
RED = 128
K = 7
K2 = 49
GC = 16
HW = 56
WPAD = 62            # padded width
NROW = 36            # rows in the padded x slab (1 pad + 34 shard + 1 pad)
XLEN = NROW * WPAD   # 2232
P = 28 * HW          # 1568 packed own pixels
HP = P // 2          # 784 per pixel half (14 rows)
CHK = HP // 2        # 392, PSUM chunk (fits one 2KB bank)
EPS49 = float(K2 * np.finfo(np.float32).eps)
ACC_DELAY = 12       # units of software pipelining for PE accumulation
PROD_DELAY = 4       # A/P-unit products emitted this many units late
PAIR_DELAY = 6       # pairsum emitted once the younger prod is this old
PATH_COUNTS = (16, 17, 16)   # (A, D, P) products per quadrant
N_PAIR = 24          # units accumulated via DVE pairsums (rest PE direct)
PATTERN = "DPA"      # repeating path pattern for the quadrant body
POOL_PAIR_EVERY = 0  # route every Nth pairsum add to Pool (0 = never)
FLUSHC_AT = 3        # unit at which the deferred phase C flushes
DMA_PLAN = 2         # prologue DMA queue assignment preset
TAIL_PATH = "D"      # path for the last 2 units of non-final quadrants
TAIL_SWAP = False    # emit tail D-products ahead of deferred flushes
P_STT = False        # Pool products via scalar_tensor_tensor (1184ns vs
                     # tensor_tensor 1651ns: TSP is costed at the default
                     # 0.6 GPSIMD efficiency, TT mult at the measured 0.42)
FINAL_CHUNKED = True # final quadrant phase C: 2 chunked copies+DMAs


def _split_multi_waits(nc, maxw=1):
    """This walrus build caps sync-wait commands per instruction; move
    excess waits onto same-engine nops inserted immediately before."""
    ctr = 0
    for fn in nc.m.functions:
        for bb in fn.blocks:
            insts = bb.instructions  # live list
            i = 0
            while i < len(insts):
                ins = insts[i]
                si = ins.sync_info
                waits = list(si.on_wait) if si is not None else []
                if len(waits) > maxw:
                    excess, keep = waits[:-maxw], waits[-maxw:]
                    for j in range(0, len(excess), maxw):
                        ctr += 1
                        nop = mybir.InstNoOp(
                            name=f"waitsplit-{ctr}",
                            engine=ins.engine,
                            bass_nofuse=True,
                            sync_info=mybir.SyncInfo(
                                on_wait=excess[j:j + maxw], on_update=[]
                            ),
                        )
                        insts.insert(i, nop)
                        i += 1
                    ins.sync_info = bass_rust.SyncInfo(
                        on_wait=keep, on_update=list(si.on_update)
                    )
                i += 1


def build_program(path_counts=None, n_pair=None, acc_delay=None,
                  prod_delay=None, pair_delay=None, pattern=None,
                  pool_pair_every=None, flushc_at=None, dma_plan=None,
                  tail_path=None, tail_swap=None, p_stt=None):
    path_counts = path_counts or PATH_COUNTS
    n_pair = n_pair if n_pair is not None else N_PAIR
    acc_delay = acc_delay if acc_delay is not None else ACC_DELAY
    prod_delay = prod_delay if prod_delay is not None else PROD_DELAY
    pair_delay = pair_delay if pair_delay is not None else PAIR_DELAY
    pattern = pattern if pattern is not None else PATTERN
    pool_pair_every = (pool_pair_every if pool_pair_every is not None
                       else POOL_PAIR_EVERY)
    flushc_at = flushc_at if flushc_at is not None else FLUSHC_AT
    dma_plan = dma_plan if dma_plan is not None else DMA_PLAN
    tail_path = tail_path if tail_path is not None else TAIL_PATH
    tail_swap = tail_swap if tail_swap is not None else TAIL_SWAP
    p_stt = p_stt if p_stt is not None else P_STT
    # which of the six ci==1 / const prologue DMAs ride the Act (scalar)
    # queue; the rest move to SP. Each scalar issue costs ~1.26us of
    # Act.SEQ time ahead of the first relu, so fewer is usually better --
    # but sync-queue crowding reorders transfers, so it's a tunable.
    _PLANS = {
        0: {"ident", "xhe1B", "xho1A", "xho1B"},
        1: {"xho1A"},
        2: set(),
        3: {"xhe1B"},
        4: set(),   # like 2, but xhe1A also moves to sync
        5: set(),   # like 2, but b2 also moves to sync
    }
    scal_xhe1a = dma_plan != 4
    scal_b2 = dma_plan != 5
    scal = _PLANS[dma_plan]
    def dma_eng(slot):
        return nc.scalar if slot in scal else nc.sync
    nc = bass.Bass("TRN2", target_bir_lowering=False, num_devices=N_CORES)

    xhe_d = nc.dram_tensor("xhe", [C, XLEN], F16, kind="ExternalInput")
    xho_d = nc.dram_tensor("xho", [C, XLEN], F16, kind="ExternalInput")
    w1_d = nc.dram_tensor("w1L", [C, RED], F16, kind="ExternalInput")
    w2_d = nc.dram_tensor("w2L", [RED, K2 * 2 * 128], F16, kind="ExternalInput")
    # b2L column K2*2 holds b1 (phase A bias) so consts load as one DMA
    b2_d = nc.dram_tensor("b2L", [128, K2 * 2 + 1], F32, kind="ExternalInput")
    id_d = nc.dram_tensor("ident", [128, 128], F16, kind="ExternalInput")
    y_d = nc.dram_tensor("y", [C, 28, HW], F32, kind="ExternalOutput")

    # per-quadrant unit schedule, LP-balanced across the four engines:
    # products nA=16 (Act extract + DVE TT), nD=16 (DVE stt from PSUM),
    # nP=17 (Act extract + Pool TT); accumulation mPE=23 direct identity
    # matmuls + 26 units pre-summed in pairs on DVE, pairsums PE-merged.
    # the last two units take the shortest dependency chain
    # (span->stt->merge) so the quadrant drain isn't gated on an Act
    # extract or a DVE pairsum; the Bresenham spread covers the rest
    CA, CD, CP = path_counts
    NBODY = K2 - 2

    def build_paths(tp):
        rem = {'A': CA, 'D': CD, 'P': CP}
        rem[tp] -= 2
        body = []
        if pattern:
            # explicit repeating pattern for the quadrant body; quota
            # counts only bound how many of each letter are consumed
            # (excess slots fall back to the largest remaining quota)
            i = 0
            while len(body) < NBODY:
                c = pattern[i % len(pattern)]
                i += 1
                if rem.get(c, 0) <= 0:
                    c = max(rem, key=lambda q: rem[q])
                rem[c] -= 1
                body.append(c)
        else:
            done = {'A': 0, 'D': 0, 'P': 0}
            for u in range(NBODY):
                pick = max(rem,
                           key=lambda c: rem[c] * (u + 1) / NBODY - done[c])
                done[pick] += 1
                body.append(pick)
        return body + [tp, tp]

    paths_mid = build_paths(tail_path)
    paths_fin = build_paths('D')
    accums = []
    pa = 0
    for u in range(K2 - 3):
        if (u + 1) * n_pair // (K2 - 3) > pa:
            pa += 1
            accums.append('PAIR')
        else:
            accums.append('PE')
    accums += ['PE', 'PE', 'PE']

    # w2 chunk tap-groups, in consumption order (t-major, host pre-reordered)
    KGRP = [(0, 13), (13, 13), (26, 13), (39, 10)]

    with tile.TileContext(nc) as tc:
        with (
            tc.tile_pool(name="sb", bufs=1) as sb,
            tc.tile_pool(name="wmp", bufs=6) as wmp,
            tc.tile_pool(name="prp", bufs=26) as prp,
            tc.tile_pool(name="osp", bufs=2) as osp,
            tc.tile_pool(name="ps", bufs=3, space="PSUM") as ps,
            tc.tile_pool(name="psacc", bufs=1, space="PSUM") as psacc,
        ):
            # DMA issue order is consumption order, and QUEUE CHOICE is
            # latency-critical: a DMA issue costs its sequencer ~850ns, so
            # the Act (scalar) queue gets only the 2 earliest-needed loads
            # (its first real op, the phase A relu, must fire ~7us in or the
            # whole DVE pipe start slips). Bulk x-slab chunks go through the
            # GPSIMD SWDGE queue -- Pool is idle all prologue and the gen
            # cost (~1us each) hides there; everything else rides sync (SP).
            # The x slabs stream in two row chunks (rows 0..20 cover phase A
            # h=0 and every h=0 view; rows 21..35 arrive later for h=1).
            RSPLIT = 21
            xhe_t = [sb.tile([128, XLEN], F16, tag=f"xhe_{ci}", name=f"xhe_{ci}")
                     for ci in range(2)]
            xho_t = [sb.tile([128, XLEN], F16, tag=f"xho_{ci}", name=f"xho_{ci}")
                     for ci in range(2)]
            nc.sync.dma_start(out=xhe_t[0][:, :RSPLIT * WPAD],
                              in_=xhe_d[0:128, :RSPLIT * WPAD])
            (nc.scalar if scal_xhe1a else nc.sync).dma_start(
                out=xhe_t[1][:, :RSPLIT * WPAD],
                in_=xhe_d[128:256, :RSPLIT * WPAD])
            # both w1 channel tiles in one DMA: out[p, ci, r] = w1[ci*128+p, r]
            w1t = sb.tile([128, 2, RED], F16, tag="w1t")
            nc.sync.dma_start(
                out=w1t[:],
                in_=w1_d[:].rearrange("(t p) r -> p t r", p=128),
            )
            w1sb = [w1t[:, ci, :] for ci in range(2)]
            b2sb = sb.tile([128, K2 * 2 + 1], F32, tag="b2")
            (nc.scalar if scal_b2 else nc.sync).dma_start(
                out=b2sb[:], in_=b2_d[:, :])
            b1sb = b2sb[:, K2 * 2:K2 * 2 + 1]

            w2sb = {}        # (t, group) -> tile
            def load_w2(t, gi, eng=None):
                k0, klen = KGRP[gi]
                w = sb.tile([RED, klen * 128], F16, tag=f"w2_{t}_{gi}")
                c0 = (t * K2 + k0) * 128
                (eng or nc.sync).dma_start(out=w[:], in_=w2_d[:, c0:c0 + klen * 128])
                w2sb[(t, gi)] = w

            load_w2(0, 0)
            idsb = sb.tile([128, 128], F16, tag="ident")
            dma_eng("ident").dma_start(out=idsb[:], in_=id_d[:, :])
            for ci in range(2):
                eng = nc.sync if ci == 0 else dma_eng("xhe1B")
                eng.dma_start(out=xhe_t[ci][:, RSPLIT * WPAD:],
                              in_=xhe_d[ci * 128:(ci + 1) * 128,
                                        RSPLIT * WPAD:])
            for ci in range(2):
                eng = nc.sync if ci == 0 else dma_eng("xho1A")
                eng.dma_start(out=xho_t[ci][:, :RSPLIT * WPAD],
                              in_=xho_d[ci * 128:(ci + 1) * 128,
                                        :RSPLIT * WPAD])
                if ci == 0:
                    load_w2(0, 1)
            for ci in range(2):
                eng = nc.sync if ci == 0 else dma_eng("xho1B")
                eng.dma_start(out=xho_t[ci][:, RSPLIT * WPAD:],
                              in_=xho_d[ci * 128:(ci + 1) * 128,
                                        RSPLIT * WPAD:])
            for gi in range(2, 4):
                load_w2(0, gi)
            for gi in range(4):
                load_w2(1, gi)

            xhe = [t[:].rearrange("p (r w) -> p r w", w=WPAD) for t in xhe_t]
            xho = [t[:].rearrange("p (r w) -> p r w", w=WPAD) for t in xho_t]

            def w2blk(t, k):
                for gi, (k0, klen) in enumerate(KGRP):
                    if k0 <= k < k0 + klen:
                        return w2sb[(t, gi)][:, (k - k0) * 128:(k - k0 + 1) * 128]
                raise AssertionError

            def xview(t, k, h):
                """x operand view [128, 14, 56] for channel tile t, tap k,
                pixel half h. Always starts at an even flat element so fp16
                rows are 4B-aligned (xho holds the odd-shifted copy)."""
                di, dj = k // K, k % K
                r0 = di + 1 + 14 * h
                if dj % 2 == 0:          # flat base 62*(di+1+14h)+dj: even dj
                    return xhe[t][:, r0:r0 + 14, dj:dj + HW]
                return xho[t][:, r0:r0 + 14, dj - 1:dj - 1 + HW]

            # Phase A: r = relu(w1' @ x + b1')  [128, P] fp16, packed pixels.
            # Center-tap (di=dj=3) view read straight from xhe (PE moving
            # operands have no alignment constraint). h=0 runs up front (its
            # x rows arrive first); h=1 is injected a few units into the
            # first quadrant so it never delays the phase B start. Relus are
            # chunked so the first span can fire off chunk 0 alone.
            r_sb = sb.tile([RED, P], F16, tag="r")

            def emit_phase_a(h):
                rps = ps.tile([128, 2, 512], F32, tag="wm", name=f"rps_{h}")
                for ci in range(2):
                    for cc in range(2):
                        r0 = 4 + 14 * h + 7 * cc
                        nc.tensor.matmul(
                            rps[:, cc, 0:CHK],
                            w1sb[ci],
                            xhe[ci][:, r0:r0 + 7, 3:3 + HW],
                            start=(ci == 0),
                            stop=(ci == 1),
                        )
                nc.scalar.activation(
                    out=r_sb[:, h * HP:(h + 1) * HP],
                    in_=rps[:, :, 0:CHK],
                    func=mybir.ActivationFunctionType.Relu,
                    bias=b1sb,
                    scale=1.0,
                )

            emit_phase_a(0)
            emit_phase_a(1)

            # Phase B: 4 quadrants of 49 tap units. The phase-C copy+DMA of
            # each quadrant is deferred a few units into the next one so the
            # Act queue is never head-of-line blocked at the boundary.
            deferred_c = [None]

            def flush_phase_c():
                if deferred_c[0] is not None:
                    deferred_c[0]()
                    deferred_c[0] = None

            for h in range(2):
                for t in range(2):
                    paths = paths_fin if (h, t) == (1, 1) else paths_mid
                    acc = psacc.tile([128, 2, 512], F32, tag="acc",
                                     name=f"acc_{h}_{t}")
                    first_pe = [True]
                    pending = []   # tiles awaiting PE identity accumulation
                    pair_q = []    # (emit_u, prod) awaiting a DVE pairsum
                    dmults = []    # deferred A-unit DVE mults (PROD_DELAY
                                   # late so the in-order DVE queue never
                                   # head-of-line blocks on Act)
                    pmults = []    # deferred P-unit Pool mults, same idea

                    def route(dest, u, accums=accums):
                        # called at product-EMISSION time so the pending /
                        # pair FIFOs respect actual availability order
                        if accums[u] == 'PE':
                            pending.append(dest)
                        else:
                            pair_q.append((u, dest))

                    pair_ctr = [0]

                    def flush_pairs(upto):
                        while len(pair_q) >= 2 and pair_q[1][0] <= upto:
                            _, p0 = pair_q.pop(0)
                            _, p1 = pair_q.pop(0)
                            psum_t = prp.tile([128, HP], F16, tag="pairsum")
                            pair_ctr[0] += 1
                            if (pool_pair_every
                                    and pair_ctr[0] % pool_pair_every == 0):
                                if p_stt:
                                    nc.gpsimd.scalar_tensor_tensor(
                                        out=psum_t[:], in0=p0[:], scalar=0.0,
                                        in1=p1[:],
                                        op0=mybir.AluOpType.add,
                                        op1=mybir.AluOpType.add,
                                    )
                                else:
                                    nc.gpsimd.tensor_tensor(
                                        psum_t[:], p0[:], p1[:],
                                        op=mybir.AluOpType.add,
                                    )
                            else:
                                nc.vector.tensor_add(psum_t[:], p0[:], p1[:])
                            pending.append(psum_t)

                    def flush_dmults(upto):
                        while dmults and dmults[0][0] <= upto:
                            uu, wmb_, dv_, xv_, dest_ = dmults.pop(0)
                            nc.vector.tensor_mul(
                                dv_[:, :, :],
                                wmb_[:].rearrange("p (r w) -> p r w", w=HW),
                                xv_,
                            )
                            route(dest_, uu)
                        while pmults and pmults[0][0] <= upto:
                            uu, wmb_, dv_, xv_, dest_ = pmults.pop(0)
                            wmv = wmb_[:].rearrange("p (r w) -> p r w", w=HW)
                            if p_stt:
                                nc.gpsimd.scalar_tensor_tensor(
                                    out=dv_[:, :, :],
                                    in0=wmv,
                                    scalar=0.0,
                                    in1=xv_,
                                    op0=mybir.AluOpType.add,
                                    op1=mybir.AluOpType.mult,
                                )
                            else:
                                nc.gpsimd.tensor_tensor(
                                    dv_[:, :, :],
                                    wmv,
                                    xv_,
                                    op=mybir.AluOpType.mult,
                                )
                            route(dest_, uu)

                    def drain_pending(keep):
                        while len(pending) > keep:
                            dst = pending.pop(0)
                            for cc in range(2):
                                nc.tensor.matmul(
                                    acc[:, cc, 0:CHK],
                                    idsb[:],
                                    dst[:, cc * CHK:(cc + 1) * CHK],
                                    start=first_pe[0],
                                    stop=False,
                                    skip_group_check=True,
                                )
                            first_pe[0] = False

                    for u in range(K2):
                        if u == flushc_at:
                            flush_phase_c()
                        k = u
                        idx = t * K2 + k
                        wmps = ps.tile([128, 2, 512], F32, tag="wm")
                        for cc in range(2):
                            nc.tensor.matmul(
                                wmps[:, cc, 0:CHK],
                                w2blk(t, k),
                                r_sb[:, h * HP + cc * CHK:
                                     h * HP + (cc + 1) * CHK],
                                start=True,
                                stop=True,
                            )
                        swap = tail_swap and u >= K2 - 2 and paths[u] == 'D'
                        if not swap:
                            flush_dmults(u - prod_delay)
                            flush_pairs(u - pair_delay)
                        # software-pipelined PE accumulation; the window must
                        # stay wider than the deferred-mult delay or an accum
                        # could be emitted before its product. Tapered near
                        # the quadrant end so the tail drain never serializes.
                        keep = acc_delay - 1 - max(0, 2 * (u - (K2 - 6)))
                        drain_pending(max(1, keep))
                        xv = xview(t, k, h)
                        dest = prp.tile([128, HP], F16, tag="prod")
                        dv = dest[:].rearrange("p (r w) -> p r w", w=HW)

                        if paths[u] == 'D':
                            # DVE straight from PSUM, bias fused (legal on
                            # HW for DVE, unlike GPSIMD)
                            nc.vector.scalar_tensor_tensor(
                                out=dv[:, :, :],
                                in0=wmps[:, :, 0:CHK],
                                scalar=b2sb[:, idx:idx + 1],
                                in1=xv,
                                op0=mybir.AluOpType.add,
                                op1=mybir.AluOpType.mult,
                            )
                            route(dest, u)
                            if swap:
                                flush_dmults(u - prod_delay)
                                flush_pairs(u - pair_delay)
                        else:
                            wmb = wmp.tile([128, HP], F16, tag="wmb")
                            nc.scalar.activation(
                                out=wmb[:],
                                in_=wmps[:, :, 0:CHK],
                                func=mybir.ActivationFunctionType.Identity,
                                bias=b2sb[:, idx:idx + 1],
                                scale=1.0,
                            )
                            if paths[u] == 'A':
                                dmults.append((u, wmb, dv, xv, dest))
                            else:
                                pmults.append((u, wmb, dv, xv, dest))

                    flush_dmults(K2)
                    flush_pairs(K2)
                    if pair_q:   # odd leftover goes straight to PE
                        pending.extend(d for _, d in pair_q)
                        pair_q.clear()
                    drain_pending(1)
                    # final merge closes the accumulation group
                    last = pending.pop(0)
                    for cc in range(2):
                        nc.tensor.matmul(
                            acc[:, cc, 0:CHK],
                            idsb[:],
                            last[:, cc * CHK:(cc + 1) * CHK],
                            start=first_pe[0],
                            stop=True,
                            skip_group_check=True,
                        )

                    def make_phase_c(acc=acc, h=h, t=t):
                        def emit():
                            osb = osp.tile([128, HP], F32, tag="osb",
                                           name=f"osb_{h}_{t}")
                            osbv = osb[:].rearrange("p (r w) -> p r w", w=HW)
                            if FINAL_CHUNKED and (h, t) == (1, 1):
                                # chunked so the first half streams out
                                # while the second merge is still running
                                for cc in range(2):
                                    nc.scalar.activation(
                                        out=osb[:, cc * CHK:(cc + 1) * CHK],
                                        in_=acc[:, cc, 0:CHK],
                                        func=mybir.ActivationFunctionType.Copy,
                                        bias=EPS49,
                                        scale=1.0,
                                    )
                                    nc.sync.dma_start(
                                        out=y_d[t * 128:(t + 1) * 128,
                                                14 * h + 7 * cc:
                                                14 * h + 7 * (cc + 1), :],
                                        in_=osbv[:, 7 * cc:7 * (cc + 1), :],
                                    )
                                return
                            nc.scalar.activation(
                                out=osb[:],
                                in_=acc[:, :, 0:CHK],
                                func=mybir.ActivationFunctionType.Copy,
                                bias=EPS49,
                                scale=1.0,
                            )
                            nc.sync.dma_start(
                                out=y_d[t * 128:(t + 1) * 128,
                                        14 * h:14 * (h + 1), :],
                                in_=osbv[:, :, :],
                            )
                        return emit

                    deferred_c[0] = make_phase_c()
            flush_phase_c()
    _split_multi_waits(nc)
    return nc


_PROGRAM = None
LAST_RESULT = None


def kernel(x, w1, b1, gamma, beta, run_mean, run_var, w2, b2):
    global _PROGRAM, LAST_RESULT
    x = np.asarray(x, dtype=np.float32)
    w1 = np.asarray(w1, dtype=np.float32)
    b1 = np.asarray(b1, dtype=np.float32)
    gamma = np.asarray(gamma, dtype=np.float32)
    beta = np.asarray(beta, dtype=np.float32)
    run_mean = np.asarray(run_mean, dtype=np.float32)
    run_var = np.asarray(run_var, dtype=np.float32)
    w2 = np.asarray(w2, dtype=np.float32)
    b2 = np.asarray(b2, dtype=np.float32)

    B = x.shape[0]
    # fold BN (eval) into the 1x1 reduce conv
    s = gamma / np.sqrt(run_var + 1e-5)
    w1p = w1 * s[:, None]
    b1p = (b1 - run_mean) * s + beta

    w1L = np.ascontiguousarray(w1p.T).astype(np.float16)  # [C, RED]

    # replicate w2 rows so the span matmul emits per-channel weights:
    # lhsT block for (channel tile t, tap k) at idx2 = t*49+k: [RED, 128],
    # col c' uses w2 row (c'//16 + 8t)*49 + k (t-major so the device can
    # stream chunks in consumption order)
    cloc = np.arange(128)
    g8 = cloc // GC
    w2L = np.empty((RED, K2 * 2 * 128), dtype=np.float16)
    b2L = np.empty((128, K2 * 2 + 1), dtype=np.float32)
    b2L[:, K2 * 2] = b1p
    w2T = w2.T.astype(np.float16)  # [RED, G*K2]
    for k in range(K2):
        for t in range(2):
            rows = (g8 + 8 * t) * K2 + k
            idx2 = t * K2 + k
            w2L[:, idx2 * 128:(idx2 + 1) * 128] = w2T[:, rows]
            b2L[:, idx2] = b2[rows]
    ident = np.eye(128, dtype=np.float16)

    xpad = np.zeros((B, C, WPAD, WPAD), dtype=np.float16)
    xpad[:, :, 3:3 + HW, 3:3 + HW] = x

    in_maps = []
    for core in range(N_CORES):
        b, half = core // 2, core % 2
        xhe = np.zeros((C, NROW, WPAD), dtype=np.float16)
        xhe[:, 1:35, :] = xpad[b, :, half * 28: half * 28 + 34, :]
        xhe = xhe.reshape(C, XLEN)
        xho = np.zeros_like(xhe)
        xho[:, :XLEN - 1] = xhe[:, 1:]
        in_maps.append({
            "xhe": xhe, "xho": xho,
            "w1L": w1L, "w2L": w2L, "b2L": b2L,
            "ident": ident,
        })

    if _PROGRAM is None:
        _PROGRAM = build_program()
    res = run_bass_kernel_spmd(_PROGRAM, in_maps, list(range(N_CORES)))
    LAST_RESULT = res

    y = np.empty((B, C, HW, HW), dtype=np.float32)
    for core in range(N_CORES):
        b, half = core // 2, core % 2
        y[b, :, half * 28:(half + 1) * 28, :] = res.results[core]["y"]
    return y

